# revision 7
# baseline (speedup 1.0000x reference)
"""Linear-chain CRF forward pass on 8 Trainium2 NeuronCores.

Reference recurrence (per batch element b):
    alpha_t[j] = x_t[j] + logsumexp_k(alpha_{t-1}[k] + trans[j,k])
    out[b] = sum_j alpha_{L_b - 1}[j]

Device formulation: exp space with a constant per-step log shift c folded
into the transition matrix:
    E_r = (Mc @ E_{r-1}) * X_r,  Mc[j,k] = exp(trans[j,k] - c),  X = exp(x)
so alpha_t = log E_r + r*c + A for a per-trajectory constant A (the
Birkhoff contraction of the positive map kills the init direction error
within a few rounds; only the scale A is unknown).

Time is cut into SEG=60 segments with starts TS[s] spread over [0, 2015);
segment s inits from its local X column at t = TS[s] - W (W=4) and runs 36
lockstep rounds (segment 0 runs the exact trajectory from t=0).  Rounds
2, 3 and 36 write their outputs into dedicated snapshot buffers that are
DMA'd out whole.  The host (float64) telescopes the per-segment offsets
A_s via class-mean log ratios where adjacent trajectories overlap
(segment 1 anchors to an exact 34-step host prefix), then rolls each
batch element's final alpha forward <=34 exact steps from the nearest
trajectory state.  Nothing on the device depends on batch_sizes.

Per-core layout (32 batch elements/core): 60 segments x 32 b = 1920
states, packed 2 segments per 128 partitions -> 960 columns, split into 4
chains [256, 256, 224, 224].  Per round each chain does one 128x128
block-diag bf16 matmul (PE, single PSUM buffer per chain) and one
elementwise PSUM combine: chains 0/1 multiply by X=exp(x) on DVE; chains
2/3 divide by exp(-x) on Pool (TensorTensor-divide runs at 0.60 gpsimd
efficiency vs 0.42 for multiply).  All recurring syncs are embedded
waits.  X streams in as fp8e4 (|x| clipped to 4 keeps it normal-range),
E state is bf16; round-1 matmuls consume the fp8 X column directly.  The
PE p-state is pre-ramped during the initial DMA window by back-to-back
matmuls on a memset tensor (the cost model keeps the high p-state across
the per-round gaps afterwards).
"""

from contextlib import ExitStack

import numpy as np

B, T, C = 256, 2048, 64
NCORES = 8
BPC = B // NCORES            # 32
SEG = 60
SPAN = 2015                  # segment starts TS[s] = round(SPAN*s/SEG)
W = 4                        # warmup rounds
TS = [round(SPAN * s / SEG) for s in range(SEG + 1)]
RSNAP = 36                   # rounds 1..RSNAP; final snapshot round
SNAPR = (2, 3, RSNAP)
TEND0 = RSNAP                # segment-0 clock is t = r
TEND = RSNAP - W             # t_end(s) = TS[s] + TEND for s >= 1
NPREF = 34                   # host-exact prefix alphas t = 0..NPREF-1
NCOLS = 960
CHAINW = [256, 256, 224, 224]
CH_OFF = [0, 256, 512, 736]
Q0 = [0, 8, 16, 23]          # first 32-col block of each chain
NCHAIN = 4
NRAMP = 26                   # PE pre-ramp matmuls
CHUNKS = [2, 6, 9, 10, 10]   # X DMA chunk sizes in rounds
XR = sum(CHUNKS)             # 37 = rounds 0..36

_CACHE = {}


def _chain_of_block(q):
    for ch in range(NCHAIN - 1, -1, -1):
        if q >= Q0[ch]:
            return ch
    raise AssertionError


def _c_step(transitions, pad_x):
    """Mean per-step growth of max_j alpha, from a short host simulation."""
    x = np.asarray(pad_x[:4], np.float64)
    tr = np.asarray(transitions, np.float64)
    a = x[:, 0, :]
    tot, n = 0.0, 0
    for t in range(1, 257):
        s = a[:, None, :] + tr[None, :, :]
        m = s.max(axis=2, keepdims=True)
        a_new = x[:, t, :] + np.log(np.exp(s - m).sum(axis=2)) + m[:, :, 0]
        tot += float((a_new.max(axis=1) - a.max(axis=1)).mean())
        n += 1
        a = a_new
    return tot / n


def _build_host_inputs(pad_x, transitions, origination, c):
    import ml_dtypes
    f8 = ml_dtypes.float8_e4m3
    mc = np.exp(np.asarray(transitions, np.float64) - c)
    wmat = np.zeros((128, 128), np.float64)
    wmat[:64, :64] = mc.T        # lhsT[k, j] = Mc[j, k]
    wmat[64:, 64:] = mc.T
    wmat = wmat.astype(ml_dtypes.bfloat16)

    xcl = np.clip(np.asarray(pad_x, np.float32), -4.0, 4.0)
    xc = xcl.reshape(NCORES, BPC, T, C)
    orig = np.asarray(origination, np.float32)

    xraw = np.empty((NCORES, 128, XR, NCOLS), np.float32)
    for s in range(SEG):
        q, half = divmod(s, 2)
        ch = _chain_of_block(q)
        off = CH_OFF[ch] + (q - Q0[ch]) * 32
        t0 = 0 if s == 0 else TS[s] - W
        tidx = np.clip(t0 + np.arange(XR), 0, T - 1)
        blk = xc[:, :, tidx, :].copy()          # (NCORES, BPC, XR, C)
        if s == 0:
            blk[:, :, 0, :] = np.clip(blk[:, :, 0, :] + orig[None, None, :],
                                      -4.0, 4.0)
        if ch >= 2:
            blk[:, :, 1:, :] *= -1.0            # divide chains: exp(-x)
        xraw[:, 64 * half:64 * half + 64, :, off:off + 32] = \
            blk.transpose(0, 3, 2, 1)
    xraw = np.exp(xraw).astype(f8)
    return xraw.reshape(NCORES, 128, XR * NCOLS), wmat


def _build_program():
    import concourse.bass as bass
    from concourse import mybir

    dt = mybir.dt
    nc = bass.Bass()
    xp = nc.declare_dram_parameter("xp", [128, XR * NCOLS], dt.float8e4,
                                   False)
    wm = nc.declare_dram_parameter("wm", [128, 128], dt.bfloat16, False)
    snaps = nc.declare_dram_parameter("snaps", [3, 128, NCOLS], dt.bfloat16,
                                      True)

    cum = np.cumsum([0] + CHUNKS)       # chunk k covers rounds cum[k]:cum[k+1]
    chunk_start_rounds = {int(cum[k]): k for k in range(1, len(CHUNKS))}

    with ExitStack() as ctx:
        def sb(name, shape, d):
            return ctx.enter_context(nc.sbuf_tensor(name, shape, d))
        wm_sb = sb("wm_sb", [128, 128], dt.bfloat16)
        rampw = sb("rampw", [128, 128], dt.bfloat16)
        xr = sb("xr", [128, XR * NCOLS], dt.float8e4)
        e = [[sb(f"e{ch}_{i}", [128, CHAINW[ch]], dt.bfloat16)
              for i in range(2)] for ch in range(NCHAIN)]
        snapb = [sb(f"snapb{d}", [128, NCOLS], dt.bfloat16) for d in range(3)]
        ps = [ctx.enter_context(
            nc.psum_tensor(f"ps{ch}", [128, CHAINW[ch]], dt.float32))
            for ch in range(NCHAIN)]
        psd = ctx.enter_context(nc.psum_tensor("psd", [128, 128], dt.float32))
        s_w = ctx.enter_context(nc.semaphore("s_w"))
        s_x = ctx.enter_context(nc.semaphore("s_x"))
        s_r = ctx.enter_context(nc.semaphore("s_r"))
        s_v = ctx.enter_context(nc.semaphore("s_v"))
        s_p = ctx.enter_context(nc.semaphore("s_p"))
        s_pe = ctx.enter_context(nc.semaphore("s_pe"))
        block = ctx.enter_context(nc.Block())

        def xsl(ch, r):
            base = r * NCOLS + CH_OFF[ch]
            return xr[:, base:base + CHAINW[ch]]

        def slot(ch, r):
            if r in SNAPR:
                return snapb[SNAPR.index(r)][:, CH_OFF[ch]:
                                             CH_OFF[ch] + CHAINW[ch]]
            return e[ch][r % 2][:]

        def mul_sem(ch):
            return (s_v, 0) if ch < 2 else (s_p, 2)

        CORD = (2, 3, 0, 1)      # pool chains first in each PE round

        @block.sync
        def _(sync):
            sync.dma_start(
                xr[:, :cum[1] * NCOLS],
                xp[:, :cum[1] * NCOLS]).then_inc(s_x, 16)
            sync.dma_start(wm_sb[:], wm[:, :]).then_inc(s_w, 16)
            for k in (1, 2):
                sync.dma_start(
                    xr[:, cum[k] * NCOLS:cum[k + 1] * NCOLS],
                    xp[:, cum[k] * NCOLS:cum[k + 1] * NCOLS]).then_inc(s_x, 16)
            for d in (0, 1):
                sync.wait_ge(s_v, 2 * SNAPR[d])
                sync.wait_ge(s_p, 2 * SNAPR[d])
                sync.dma_start(snaps[d], snapb[d][:])
            for k in (3, 4):
                sync.dma_start(
                    xr[:, cum[k] * NCOLS:cum[k + 1] * NCOLS],
                    xp[:, cum[k] * NCOLS:cum[k + 1] * NCOLS]).then_inc(s_x, 16)
            sync.wait_ge(s_v, 2 * RSNAP)
            sync.dma_start(snaps[2, :, :512], snapb[2][:, :512])
            sync.wait_ge(s_p, 2 * RSNAP)
            sync.dma_start(snaps[2, :, 512:], snapb[2][:, 512:])

        @block.tensor
        def _(tensor):
            tensor.wait_ge(s_r, 1)
            for _ in range(NRAMP):
                nc.tensor.matmul(psd[:], rampw[:], rampw[:],
                                 start=True, stop=True)
            tensor.wait_ge(s_w, 16)
            tensor.wait_ge(s_x, 16)
            for ch in CORD:
                nc.tensor.matmul(ps[ch][:], wm_sb[:], xsl(ch, 0),
                                 start=True, stop=True).then_inc(s_pe, 1)
            for r in range(2, RSNAP + 1):
                for ch in CORD:
                    sem, cb = mul_sem(ch)
                    mm = nc.tensor.matmul(ps[ch][:], wm_sb[:],
                                          slot(ch, r - 1),
                                          start=True, stop=True)
                    mm._wait_ge(sem, 2 * (r - 2) + (ch - cb) + 1)
                    mm.then_inc(s_pe, 1)

        @block.vector
        def _(vector):
            nc.vector.memset(rampw[:], 1.0).then_inc(s_r, 1)
            for r in range(1, RSNAP + 1):
                if r in chunk_start_rounds:
                    vector.wait_ge(s_x, 16 * (chunk_start_rounds[r] + 1))
                for ch in (0, 1):
                    mul = nc.vector.tensor_mul(slot(ch, r), ps[ch][:],
                                               xsl(ch, r))
                    mul._wait_ge(s_pe, 4 * (r - 1) + CORD.index(ch) + 1)
                    mul.then_inc(s_v, 1)

        @block.gpsimd
        def _(gpsimd):
            for r in range(1, RSNAP + 1):
                if r in chunk_start_rounds:
                    gpsimd.wait_ge(s_x, 16 * (chunk_start_rounds[r] + 1))
                for ch in (2, 3):
                    div = nc.gpsimd.tensor_tensor(slot(ch, r), ps[ch][:],
                                                  xsl(ch, r),
                                                  mybir.AluOpType.divide)
                    div._wait_ge(s_pe, 4 * (r - 1) + CORD.index(ch) + 1)
                    div.then_inc(s_p, 1)

    return nc


def _seg_cols(ls_d, s):
    """(64, 32) class x batch block of a (128, NCOLS) dump for segment s."""
    q, half = divmod(s, 2)
    ch = _chain_of_block(q)
    off = CH_OFF[ch] + (q - Q0[ch]) * 32
    return ls_d[64 * half:64 * half + 64, off:off + 32]


def _lse_step(a, x_t, trans):
    sc = a[:, None, :] + trans[None, :, :]
    m = sc.max(axis=2, keepdims=True)
    return x_t + np.log(np.exp(sc - m).sum(axis=2)) + m[:, :, 0]


def kernel(pad_x, transitions, origination, batch_sizes):
    from concourse.bass_utils import run_bass_kernel_spmd

    pad_x = np.asarray(pad_x)
    transitions = np.asarray(transitions)
    origination = np.asarray(origination)
    batch_sizes = np.asarray(batch_sizes)

    c = _c_step(transitions, pad_x)
    xraw, wmat = _build_host_inputs(pad_x, transitions, origination, c)

    if "nc" not in _CACHE:
        _CACHE["nc"] = _build_program()
    nc = _CACHE["nc"]

    in_maps = [{"xp": xraw[i], "wm": wmat} for i in range(NCORES)]
    out = run_bass_kernel_spmd(nc, in_maps, list(range(NCORES)))

    # ---- host post-processing (float64) ----
    x = np.asarray(pad_x, np.float64)
    trans = np.asarray(transitions, np.float64)
    orig = np.asarray(origination, np.float64)
    bs = np.asarray(batch_sizes).astype(np.int64)

    # exact prefix alphas t = 0..NPREF-1
    alpha_exact = np.empty((NPREF, B, C))
    a = x[:, 0, :] + orig[None, :]
    alpha_exact[0] = a
    for t in range(1, NPREF):
        a = _lse_step(a, x[:, t, :], trans)
        alpha_exact[t] = a

    ls = np.empty((NCORES, 3, 128, NCOLS))
    for i in range(NCORES):
        ls[i] = np.log(np.maximum(
            np.asarray(out.results[i]["snaps"], np.float64), 1e-300))

    # stitch offsets A[s] per global b; segment 1 anchors to the exact
    # prefix at t = TS[1] - W + 3 (its round-3 snapshot)
    A = np.zeros((SEG, B))
    for i in range(NCORES):
        bsl = slice(i * BPC, (i + 1) * BPC)
        cur = _seg_cols(ls[i, SNAPR.index(3)], 1)
        A[1, bsl] = (alpha_exact[TS[1] - W + 3, bsl].T
                     - (cur + 3 * c)).mean(axis=0)
        for s in range(2, SEG):
            rs = RSNAP - (TS[s] - TS[s - 1])            # 2 or 3
            prev = _seg_cols(ls[i, 2], s - 1)
            cur = _seg_cols(ls[i, SNAPR.index(rs)], s)
            A[s, bsl] = A[s - 1, bsl] + \
                ((prev + RSNAP * c) - (cur + rs * c)).mean(axis=0)

    # roll sources sorted by time: exact prefix, then trajectory ends
    src_t = list(range(NPREF))
    src_alpha = [alpha_exact[t] for t in range(NPREF)]
    ends = np.empty((SEG, B, C))
    for i in range(NCORES):
        for s in range(SEG):
            ends[s, i * BPC:(i + 1) * BPC] = _seg_cols(ls[i, 2], s).T
    src_t.append(TEND0)                                 # segment 0: t = RSNAP
    src_alpha.append(ends[0] + RSNAP * c)
    for s in range(1, SEG):
        src_t.append(TS[s] + TEND)
        src_alpha.append(ends[s] + RSNAP * c + A[s][:, None])
    src_t = np.asarray(src_t)

    tstar = bs - 1
    idx = np.searchsorted(src_t, tstar, side="right") - 1
    t0 = src_t[idx]
    av = np.stack([src_alpha[idx[b]][b] for b in range(B)])   # (B, C)
    kmax = int((tstar - t0).max())
    for kk in range(1, kmax + 1):
        act = np.nonzero(t0 + kk <= tstar)[0]
        if len(act) == 0:
            break
        tb = t0[act] + kk
        av[act] = _lse_step(av[act], x[act, tb, :], trans)
    return av.sum(axis=1).astype(np.float32)


# revision 8
# speedup vs baseline: 1.0091x; 1.0091x over previous
"""Linear-chain CRF forward pass on 8 Trainium2 NeuronCores.

Reference recurrence (per batch element b):
    alpha_t[j] = x_t[j] + logsumexp_k(alpha_{t-1}[k] + trans[j,k])
    out[b] = sum_j alpha_{L_b - 1}[j]

Device formulation: exp space with a constant per-step log shift c folded
into the transition matrix:
    E_r = (Mc @ E_{r-1}) * X_r,  Mc[j,k] = exp(trans[j,k] - c),  X = exp(x)
so alpha_t = log E_r + r*c + A for a per-trajectory constant A (the
Birkhoff contraction of the positive map kills the init direction error
within a few rounds; only the scale A is unknown).

Time is cut into SEG=60 segments with starts TS[s] spread over [0, 2015);
segment s inits from its local X column at t = TS[s] - W (W=4) and runs 36
lockstep rounds (segment 0 runs the exact trajectory from t=0).  Rounds
2, 3 and 36 write their outputs into dedicated snapshot buffers that are
DMA'd out whole.  The host (float64) telescopes the per-segment offsets
A_s via class-mean log ratios where adjacent trajectories overlap
(segment 1 anchors to an exact 34-step host prefix), then rolls each
batch element's final alpha forward <=34 exact steps from the nearest
trajectory state.  Nothing on the device depends on batch_sizes.

Per-core layout (32 batch elements/core): 60 segments x 32 b = 1920
states, packed 2 segments per 128 partitions -> 960 columns, split into 4
chains [256, 256, 224, 224].  Per round each chain does one 128x128
block-diag bf16 matmul (PE, single PSUM buffer per chain) and one
elementwise PSUM combine: chains 0/1 multiply by X=exp(x) on DVE; chains
2/3 divide by exp(-x) on Pool (TensorTensor-divide runs at 0.60 gpsimd
efficiency vs 0.42 for multiply).  All recurring syncs are embedded
waits.  X streams in as fp8e4 (|x| clipped to 4 keeps it normal-range),
E state is bf16; round-1 matmuls consume the fp8 X column directly.  The
PE p-state is pre-ramped during the initial DMA window by back-to-back
matmuls on a memset tensor (the cost model keeps the high p-state across
the per-round gaps afterwards).
"""

from contextlib import ExitStack

import numpy as np

B, T, C = 256, 2048, 64
NCORES = 8
BPC = B // NCORES            # 32
SEG = 60
SPAN = 2015                  # segment starts TS[s] = round(SPAN*s/SEG)
W = 4                        # warmup rounds
TS = [round(SPAN * s / SEG) for s in range(SEG + 1)]
RSNAP = 36                   # rounds 1..RSNAP; final snapshot round
SNAPR = (2, 3, RSNAP)
TEND0 = RSNAP                # segment-0 clock is t = r
TEND = RSNAP - W             # t_end(s) = TS[s] + TEND for s >= 1
NPREF = 34                   # host-exact prefix alphas t = 0..NPREF-1
NCOLS = 960
CHAINW = [256, 256, 224, 224]
CH_OFF = [0, 256, 512, 736]
Q0 = [0, 8, 16, 23]          # first 32-col block of each chain
NCHAIN = 4
NRAMP = 26                   # PE pre-ramp matmuls
CHUNKS = [2, 6, 9, 10, 10]   # X DMA chunk sizes in rounds
XR = sum(CHUNKS)             # 37 = rounds 0..36

_CACHE = {}


def _chain_of_block(q):
    for ch in range(NCHAIN - 1, -1, -1):
        if q >= Q0[ch]:
            return ch
    raise AssertionError


def _c_step(transitions, pad_x):
    """Mean per-step growth of max_j alpha, from a short host simulation."""
    x = np.asarray(pad_x[:4], np.float64)
    tr = np.asarray(transitions, np.float64)
    a = x[:, 0, :]
    tot, n = 0.0, 0
    for t in range(1, 257):
        s = a[:, None, :] + tr[None, :, :]
        m = s.max(axis=2, keepdims=True)
        a_new = x[:, t, :] + np.log(np.exp(s - m).sum(axis=2)) + m[:, :, 0]
        tot += float((a_new.max(axis=1) - a.max(axis=1)).mean())
        n += 1
        a = a_new
    return tot / n


def _build_host_inputs(pad_x, transitions, origination, c):
    import ml_dtypes
    f8 = ml_dtypes.float8_e4m3
    mc = np.exp(np.asarray(transitions, np.float64) - c)
    wmat = np.zeros((128, 128), np.float64)
    wmat[:64, :64] = mc.T        # lhsT[k, j] = Mc[j, k]
    wmat[64:, 64:] = mc.T
    wmat = wmat.astype(ml_dtypes.bfloat16)

    xcl = np.clip(np.asarray(pad_x, np.float32), -4.0, 4.0)
    xc = xcl.reshape(NCORES, BPC, T, C)
    orig = np.asarray(origination, np.float32)

    xraw = np.empty((NCORES, 128, XR, NCOLS), np.float32)
    for s in range(SEG):
        q, half = divmod(s, 2)
        ch = _chain_of_block(q)
        off = CH_OFF[ch] + (q - Q0[ch]) * 32
        t0 = 0 if s == 0 else TS[s] - W
        tidx = np.clip(t0 + np.arange(XR), 0, T - 1)
        blk = xc[:, :, tidx, :].copy()          # (NCORES, BPC, XR, C)
        if s == 0:
            blk[:, :, 0, :] = np.clip(blk[:, :, 0, :] + orig[None, None, :],
                                      -4.0, 4.0)
        if ch >= 2:
            blk[:, :, 1:, :] *= -1.0            # divide chains: exp(-x)
        xraw[:, 64 * half:64 * half + 64, :, off:off + 32] = \
            blk.transpose(0, 3, 2, 1)
    xraw = np.exp(xraw).astype(f8)
    return xraw.reshape(NCORES, 128, XR * NCOLS), wmat


def _build_program():
    import concourse.bass as bass
    from concourse import mybir

    dt = mybir.dt
    nc = bass.Bass()
    xp = nc.declare_dram_parameter("xp", [128, XR * NCOLS], dt.float8e4,
                                   False)
    wm = nc.declare_dram_parameter("wm", [128, 128], dt.bfloat16, False)
    snaps = nc.declare_dram_parameter("snaps", [3, 128, NCOLS], dt.bfloat16,
                                      True)

    cum = np.cumsum([0] + CHUNKS)       # chunk k covers rounds cum[k]:cum[k+1]
    chunk_start_rounds = {int(cum[k]): k for k in range(1, len(CHUNKS))}

    with ExitStack() as ctx:
        def sb(name, shape, d):
            return ctx.enter_context(nc.sbuf_tensor(name, shape, d))
        wm_sb = sb("wm_sb", [128, 128], dt.bfloat16)
        rampw = sb("rampw", [128, 128], dt.bfloat16)
        xr = sb("xr", [128, XR * NCOLS], dt.float8e4)
        e = [[sb(f"e{ch}_{i}", [128, CHAINW[ch]], dt.bfloat16)
              for i in range(2)] for ch in range(NCHAIN)]
        snapb = [sb(f"snapb{d}", [128, NCOLS], dt.bfloat16) for d in range(3)]
        ps = [ctx.enter_context(
            nc.psum_tensor(f"ps{ch}", [128, CHAINW[ch]], dt.float32))
            for ch in range(NCHAIN)]
        psd = ctx.enter_context(nc.psum_tensor("psd", [128, 128], dt.float32))
        s_w = ctx.enter_context(nc.semaphore("s_w"))
        s_x = ctx.enter_context(nc.semaphore("s_x"))
        s_r = ctx.enter_context(nc.semaphore("s_r"))
        s_v = ctx.enter_context(nc.semaphore("s_v"))
        s_p = ctx.enter_context(nc.semaphore("s_p"))
        s_pe = ctx.enter_context(nc.semaphore("s_pe"))
        block = ctx.enter_context(nc.Block())

        def xsl(ch, r):
            base = r * NCOLS + CH_OFF[ch]
            return xr[:, base:base + CHAINW[ch]]

        def slot(ch, r):
            if r in SNAPR:
                return snapb[SNAPR.index(r)][:, CH_OFF[ch]:
                                             CH_OFF[ch] + CHAINW[ch]]
            return e[ch][r % 2][:]

        def mul_sem(ch):
            return (s_v, 0) if ch < 2 else (s_p, 2)

        CORD = (0, 1, 2, 3)

        @block.sync
        def _(sync):
            sync.dma_start(
                xr[:, :cum[1] * NCOLS],
                xp[:, :cum[1] * NCOLS]).then_inc(s_x, 16)
            sync.dma_start(wm_sb[:], wm[:, :]).then_inc(s_w, 16)
            for k in (1, 2):
                sync.dma_start(
                    xr[:, cum[k] * NCOLS:cum[k + 1] * NCOLS],
                    xp[:, cum[k] * NCOLS:cum[k + 1] * NCOLS]).then_inc(s_x, 16)
            for d in (0, 1):
                sync.wait_ge(s_v, 2 * SNAPR[d])
                sync.wait_ge(s_p, 2 * SNAPR[d])
                sync.dma_start(snaps[d], snapb[d][:])
            for k in (3, 4):
                sync.dma_start(
                    xr[:, cum[k] * NCOLS:cum[k + 1] * NCOLS],
                    xp[:, cum[k] * NCOLS:cum[k + 1] * NCOLS]).then_inc(s_x, 16)
            sync.wait_ge(s_v, 2 * RSNAP)
            sync.dma_start(snaps[2, :, :512], snapb[2][:, :512])
            sync.wait_ge(s_p, 2 * RSNAP)
            sync.dma_start(snaps[2, :, 512:], snapb[2][:, 512:])

        @block.tensor
        def _(tensor):
            tensor.wait_ge(s_r, 1)
            for _ in range(NRAMP):
                nc.tensor.matmul(psd[:], rampw[:], rampw[:],
                                 start=True, stop=True)
            tensor.wait_ge(s_w, 16)
            tensor.wait_ge(s_x, 16)
            for ch in CORD:
                nc.tensor.matmul(ps[ch][:], wm_sb[:], xsl(ch, 0),
                                 start=True, stop=True).then_inc(s_pe, 1)
            for r in range(2, RSNAP + 1):
                for ch in CORD:
                    sem, cb = mul_sem(ch)
                    mm = nc.tensor.matmul(ps[ch][:], wm_sb[:],
                                          slot(ch, r - 1),
                                          start=True, stop=True)
                    mm._wait_ge(sem, 2 * (r - 2) + (ch - cb) + 1)
                    mm.then_inc(s_pe, 1)

        @block.vector
        def _(vector):
            nc.vector.memset(rampw[:], 1.0).then_inc(s_r, 1)
            for r in range(1, RSNAP + 1):
                if r in chunk_start_rounds:
                    vector.wait_ge(s_x, 16 * (chunk_start_rounds[r] + 1))
                for ch in (0, 1):
                    mul = nc.vector.tensor_mul(slot(ch, r), ps[ch][:],
                                               xsl(ch, r))
                    mul._wait_ge(s_pe, 4 * (r - 1) + CORD.index(ch) + 1)
                    mul.then_inc(s_v, 1)

        @block.gpsimd
        def _(gpsimd):
            for r in range(1, RSNAP + 1):
                if r in chunk_start_rounds:
                    gpsimd.wait_ge(s_x, 16 * (chunk_start_rounds[r] + 1))
                for ch in (2, 3):
                    div = nc.gpsimd.tensor_tensor(slot(ch, r), ps[ch][:],
                                                  xsl(ch, r),
                                                  mybir.AluOpType.divide)
                    div._wait_ge(s_pe, 4 * (r - 1) + CORD.index(ch) + 1)
                    div.then_inc(s_p, 1)

    return nc


def _seg_cols(ls_d, s):
    """(64, 32) class x batch block of a (128, NCOLS) dump for segment s."""
    q, half = divmod(s, 2)
    ch = _chain_of_block(q)
    off = CH_OFF[ch] + (q - Q0[ch]) * 32
    return ls_d[64 * half:64 * half + 64, off:off + 32]


def _lse_step(a, x_t, trans):
    sc = a[:, None, :] + trans[None, :, :]
    m = sc.max(axis=2, keepdims=True)
    return x_t + np.log(np.exp(sc - m).sum(axis=2)) + m[:, :, 0]


def kernel(pad_x, transitions, origination, batch_sizes):
    from concourse.bass_utils import run_bass_kernel_spmd

    pad_x = np.asarray(pad_x)
    transitions = np.asarray(transitions)
    origination = np.asarray(origination)
    batch_sizes = np.asarray(batch_sizes)

    c = _c_step(transitions, pad_x)
    xraw, wmat = _build_host_inputs(pad_x, transitions, origination, c)

    if "nc" not in _CACHE:
        _CACHE["nc"] = _build_program()
    nc = _CACHE["nc"]

    in_maps = [{"xp": xraw[i], "wm": wmat} for i in range(NCORES)]
    out = run_bass_kernel_spmd(nc, in_maps, list(range(NCORES)))

    # ---- host post-processing (float64) ----
    x = np.asarray(pad_x, np.float64)
    trans = np.asarray(transitions, np.float64)
    orig = np.asarray(origination, np.float64)
    bs = np.asarray(batch_sizes).astype(np.int64)

    # exact prefix alphas t = 0..NPREF-1
    alpha_exact = np.empty((NPREF, B, C))
    a = x[:, 0, :] + orig[None, :]
    alpha_exact[0] = a
    for t in range(1, NPREF):
        a = _lse_step(a, x[:, t, :], trans)
        alpha_exact[t] = a

    ls = np.empty((NCORES, 3, 128, NCOLS))
    for i in range(NCORES):
        ls[i] = np.log(np.maximum(
            np.asarray(out.results[i]["snaps"], np.float64), 1e-300))

    # stitch offsets A[s] per global b; segment 1 anchors to the exact
    # prefix at t = TS[1] - W + 3 (its round-3 snapshot)
    A = np.zeros((SEG, B))
    for i in range(NCORES):
        bsl = slice(i * BPC, (i + 1) * BPC)
        cur = _seg_cols(ls[i, SNAPR.index(3)], 1)
        A[1, bsl] = (alpha_exact[TS[1] - W + 3, bsl].T
                     - (cur + 3 * c)).mean(axis=0)
        for s in range(2, SEG):
            rs = RSNAP - (TS[s] - TS[s - 1])            # 2 or 3
            prev = _seg_cols(ls[i, 2], s - 1)
            cur = _seg_cols(ls[i, SNAPR.index(rs)], s)
            A[s, bsl] = A[s - 1, bsl] + \
                ((prev + RSNAP * c) - (cur + rs * c)).mean(axis=0)

    # roll sources sorted by time: exact prefix, then trajectory ends
    src_t = list(range(NPREF))
    src_alpha = [alpha_exact[t] for t in range(NPREF)]
    ends = np.empty((SEG, B, C))
    for i in range(NCORES):
        for s in range(SEG):
            ends[s, i * BPC:(i + 1) * BPC] = _seg_cols(ls[i, 2], s).T
    src_t.append(TEND0)                                 # segment 0: t = RSNAP
    src_alpha.append(ends[0] + RSNAP * c)
    for s in range(1, SEG):
        src_t.append(TS[s] + TEND)
        src_alpha.append(ends[s] + RSNAP * c + A[s][:, None])
    src_t = np.asarray(src_t)

    tstar = bs - 1
    idx = np.searchsorted(src_t, tstar, side="right") - 1
    t0 = src_t[idx]
    av = np.stack([src_alpha[idx[b]][b] for b in range(B)])   # (B, C)
    kmax = int((tstar - t0).max())
    for kk in range(1, kmax + 1):
        act = np.nonzero(t0 + kk <= tstar)[0]
        if len(act) == 0:
            break
        tb = t0[act] + kk
        av[act] = _lse_step(av[act], x[act, tb, :], trans)
    return av.sum(axis=1).astype(np.float32)
